# revision 4
# baseline (speedup 1.0000x reference)
"""Additive (Bahdanau) attention on 8 Trainium2 NeuronCores.

reference:
    q_proj  = query @ W_query.T                     # (B, H)
    k_proj  = einsum('bsh,oh->bso', keys, W_key)    # (B, S, O)
    scores  = einsum('bso,o->bs', tanh(q_proj[:,None,:] + k_proj), V[0])
    weights = softmax(scores, axis=-1)              # (B, S)
    context = einsum('bs,bsh->bh', weights, keys)   # (B, H)

Sharding: data-parallel over batch, 4 batches per core, no collectives.

Per-core kernel (all matmuls in float32r -> full PE rate at N=512):
  pass 1 (scores): k_projT chunks [o=128, s=512] = sum_hc W_keyT[hc,oc].T @ keysT[hc, s]
                   ACT computes tanh(psum + q_proj bias) fused, then
                   scores[1, s] accumulates V.T @ tanh-tile over o-chunks.
  softmax: free-dim max / fused exp+sum on ACT / reciprocal / scale.
  pass 2 (context): weights are written to DRAM (they are an output anyway),
                   read back transposed [s=128, 16], then
                   context[1, h] = sum_sc wT[:, sc].T @ keys[sc*128:.., h].
"""

import numpy as np

import concourse.bacc as bacc
import concourse.mybir as mybir
import concourse.tile as tile
from concourse.bass_utils import run_bass_kernel_spmd

N_CORES = 8
B_GLOBAL, S, H = 32, 2048, 1024
B = B_GLOBAL // N_CORES          # local batches per core
P = 128                          # partitions
HC = H // P                      # 8 contraction chunks
OC = H // P                      # 8 output-hidden chunks
SB = 512                         # s-block (matmul free dim)
NSB = S // SB                    # 4 s-blocks
SC = S // P                      # 16 s-chunks for context contraction

F32 = mybir.dt.float32
F32R = mybir.dt.float32r


def _r(ap):
    return ap.bitcast(F32R)


def _build():
    nc = bacc.Bacc("TRN2", target_bir_lowering=False, debug=False,
                   num_devices=N_CORES)

    keysT = nc.dram_tensor("keysT", [B, H, S], F32, kind="ExternalInput").ap()
    keysN = nc.dram_tensor("keysN", [B, S, H], F32, kind="ExternalInput").ap()
    wkT = nc.dram_tensor("wkT", [H, H], F32, kind="ExternalInput").ap()
    wqT = nc.dram_tensor("wqT", [H, H], F32, kind="ExternalInput").ap()
    qT = nc.dram_tensor("qT", [H, B], F32, kind="ExternalInput").ap()
    vcol = nc.dram_tensor("vcol", [H, 1], F32, kind="ExternalInput").ap()

    ctx_out = nc.dram_tensor("ctx_out", [B, H], F32, kind="ExternalOutput").ap()
    w_out = nc.dram_tensor("w_out", [B, S], F32, kind="ExternalOutput").ap()

    with tile.TileContext(nc) as tc:
        with (
            tc.tile_pool(name="singles", bufs=1) as singles,
            tc.tile_pool(name="kt", bufs=2) as kt_pool,
            tc.tile_pool(name="tt", bufs=3) as t_pool,
            tc.tile_pool(name="scores", bufs=2) as scores_pool,
            tc.tile_pool(name="wrow", bufs=3) as w_pool,
            tc.tile_pool(name="small", bufs=8) as small,
            tc.tile_pool(name="wt", bufs=2) as wt_pool,
            tc.tile_pool(name="kn", bufs=3) as kn_pool,
            tc.tile_pool(name="csb", bufs=2) as csb_pool,
            tc.tile_pool(name="psk", bufs=3, space="PSUM") as psk_pool,
            tc.tile_pool(name="pss", bufs=2, space="PSUM") as pss_pool,
            tc.tile_pool(name="psq", bufs=1, space="PSUM") as psq_pool,
            tc.tile_pool(name="psc", bufs=1, space="PSUM") as psc_pool,
        ):
            # ---- load weights / query / V ----
            wk_sb = singles.tile([P, HC, H], F32R)
            nc.sync.dma_start(out=wk_sb, in_=wkT.bitcast(F32R).rearrange("(hc p) o -> p hc o", p=P))
            wq_sb = singles.tile([P, HC, H], F32R)
            nc.sync.dma_start(out=wq_sb, in_=wqT.bitcast(F32R).rearrange("(hc p) o -> p hc o", p=P))
            qt_sb = singles.tile([P, HC, B], F32R)
            nc.sync.dma_start(out=qt_sb, in_=qT.bitcast(F32R).rearrange("(hc p) b -> p hc b", p=P))
            v_sb = singles.tile([P, OC], F32R)
            nc.sync.dma_start(out=v_sb, in_=vcol.bitcast(F32R).rearrange("(oc p) one -> p (oc one)", p=P))

            # ---- q_proj: q_bias[p, oc, b] = (query @ W_query.T)[b, oc*128+p] ----
            q_bias = singles.tile([P, OC, B], F32)
            for oc in range(OC):
                psq = psq_pool.tile([P, B], F32)
                osl = slice(oc * P, (oc + 1) * P)
                for hc in range(HC):
                    nc.tensor.matmul(psq, wq_sb[:, hc, osl], qt_sb[:, hc, :],
                                     start=(hc == 0), stop=(hc == HC - 1))
                nc.vector.tensor_copy(q_bias[:, oc, :], psq)

            for b in range(B):
                # ======== pass 1: scores for batch b ========
                scores_row = scores_pool.tile([1, S], F32)
                for sblk in range(NSB):
                    ssl = slice(sblk * SB, (sblk + 1) * SB)
                    kt = kt_pool.tile([P, HC, SB], F32R)
                    nc.sync.dma_start(
                        out=kt,
                        in_=keysT.bitcast(F32R)[b].rearrange("(hc p) s -> p hc s", p=P)[:, :, ssl])
                    pss = pss_pool.tile([1, SB], F32)
                    for oc in range(OC):
                        osl = slice(oc * P, (oc + 1) * P)
                        psk = psk_pool.tile([P, SB], F32)
                        for hc in range(HC):
                            nc.tensor.matmul(psk, wk_sb[:, hc, osl], kt[:, hc, :],
                                             start=(hc == 0), stop=(hc == HC - 1))
                        tt = t_pool.tile([P, SB], F32R)
                        nc.scalar.activation(tt, psk, mybir.ActivationFunctionType.Tanh,
                                             bias=q_bias[:, oc, b:b + 1])
                        nc.tensor.matmul(pss, v_sb[:, oc:oc + 1], tt,
                                         start=(oc == 0), stop=(oc == OC - 1))
                    nc.vector.tensor_copy(scores_row[0:1, ssl], pss)

                # ======== softmax over S (single lane) ========
                negm = small.tile([1, 1], F32)
                nc.vector.tensor_reduce(negm, scores_row, mybir.AxisListType.X,
                                        mybir.AluOpType.max, negate=True)
                expw = w_pool.tile([1, S], F32)
                lsum = small.tile([1, 1], F32)
                nc.scalar.activation(expw, scores_row, mybir.ActivationFunctionType.Exp,
                                     bias=negm, accum_out=lsum)
                rl = small.tile([1, 1], F32)
                nc.vector.reciprocal(rl, lsum)
                wn = w_pool.tile([1, S], F32)
                nc.vector.tensor_scalar_mul(wn, expw, rl)
                nc.sync.dma_start(out=w_out[b:b + 1, :], in_=wn)

                # ======== pass 2: context for batch b ========
                wt = wt_pool.tile([P, SC], F32R)
                nc.sync.dma_start(
                    out=wt, in_=w_out.bitcast(F32R)[b:b + 1, :].rearrange("one (sc p) -> (one p) sc", p=P))
                psc0 = psc_pool.tile([1, SB], F32)
                psc1 = psc_pool.tile([1, SB], F32)
                for sc in range(SC):
                    kn = kn_pool.tile([P, H], F32R)
                    nc.sync.dma_start(out=kn, in_=keysN.bitcast(F32R)[b, sc * P:(sc + 1) * P, :])
                    nc.tensor.matmul(psc0, wt[:, sc:sc + 1], kn[:, 0:SB],
                                     start=(sc == 0), stop=(sc == SC - 1))
                    nc.tensor.matmul(psc1, wt[:, sc:sc + 1], kn[:, SB:H],
                                     start=(sc == 0), stop=(sc == SC - 1))
                csb = csb_pool.tile([1, H], F32)
                nc.vector.tensor_copy(csb[0:1, 0:SB], psc0)
                nc.vector.tensor_copy(csb[0:1, SB:H], psc1)
                nc.sync.dma_start(out=ctx_out[b:b + 1, :], in_=csb)

    nc.compile()
    return nc


_compiled_nc = None


def _in_maps(query, keys, W_query, W_key, V):
    query = np.ascontiguousarray(np.asarray(query, dtype=np.float32))
    keys = np.ascontiguousarray(np.asarray(keys, dtype=np.float32))
    wkT = np.ascontiguousarray(np.asarray(W_key, dtype=np.float32).T)
    wqT = np.ascontiguousarray(np.asarray(W_query, dtype=np.float32).T)
    vcol = np.ascontiguousarray(np.asarray(V, dtype=np.float32).reshape(H, 1))
    maps = []
    for c in range(N_CORES):
        kb = keys[c * B:(c + 1) * B]
        maps.append({
            "keysT": np.ascontiguousarray(kb.transpose(0, 2, 1)),
            "keysN": np.ascontiguousarray(kb),
            "wkT": wkT,
            "wqT": wqT,
            "qT": np.ascontiguousarray(query[c * B:(c + 1) * B].T),
            "vcol": vcol,
        })
    return maps


def kernel(query, keys, W_query, W_key, V, _trace=False, _trace_kwargs=None):
    global _compiled_nc
    if _compiled_nc is None:
        _compiled_nc = _build()
    maps = _in_maps(query, keys, W_query, W_key, V)
    out = run_bass_kernel_spmd(_compiled_nc, maps, list(range(N_CORES)),
                               trace=_trace, **(_trace_kwargs or {}))
    res = out.results
    context = np.concatenate([res[c]["ctx_out"] for c in range(N_CORES)], axis=0)
    weights = np.concatenate([res[c]["w_out"] for c in range(N_CORES)], axis=0)
    if _trace:
        return (context, weights), out
    return context, weights
